# revision 2
# baseline (speedup 1.0000x reference)
"""Trainium2 Bass kernel for nn_BubblePredictor (GRU recurrence + linear head).

Full problem: history [1024, 2048, 12] fp32, torch-GRUCell math (bias-free)
with H=512, per-step 2-unit head. Returns (logits [1024, 2048, 2], h [1024, 512]).

Sharding (data-parallel, per the problem's hint): batch split across the 8
NeuronCores (128 rows each); GRU + head weights replicated; the time
recurrence stays local per shard. Inside kernel(): inputs are sharded /
layout-prepped on host, the Bass kernel runs via run_bass_kernel_spmd on
cores 0-7, outputs are gathered and reassembled.

Per-core kernel design (batch-major state, bf16 matmuls, fp32 gates/state):
  - h [128, 512] fp32 master in SBUF; a bf16 transposed copy hT (4 chunks of
    [128,128]) serves as the matmul stationary operand each step.
  - Gate pre-activations via PE into PSUM: gh = h @ w_hh.T as 4 K-chunk
    accumulating matmuls per gate bank (N=512), with gi = x_t @ w_ih.T
    accumulated on top by the PE itself for the r/z banks (K=12 matmuls whose
    stationary is a pre-transposed x slice) — the elementwise adds are free.
  - ACT: r = sigmoid(bank_r), 1-z = sigmoid(-bank_z) (scale=-1), n = tanh(d).
  - DVE: c = r*gh_n, d = c+gi_n, e = n-h, f = (1-z)*e, h' = h+f (fp32),
    bf16 cast; PE transposes h' (4x [128,128] via identity) and a copy
    rebuilds hT for the next step.
  - Head: logit_t = h_{t+1} @ w_out.T runs one step delayed (so it reuses the
    just-built hT), accumulating 256 steps into one PSUM bank before a bulk
    copy into the SBUF-resident logit buffer (written to DRAM once at end).
  - x is host-pre-transposed and bf16-cast to xt[replica, t, batch] (3
    replicas at partitions 0/32/64 so the three gi matmuls can pack into
    distinct PE row groups); w_hh.T/w_out.T are pre-chunked into the moving
    operand layout; all hardcoded for B=1024, T=2048, I=12, H=512.
"""

import sys

sys.path.insert(0, "/opt/trn_rl_repo")

from contextlib import ExitStack

import numpy as np
import ml_dtypes

import concourse.mybir as mybir
import concourse.tile as tile
from concourse import bacc
from concourse.bass_utils import run_bass_kernel_spmd
from concourse.masks import make_identity

F32 = mybir.dt.float32
BF16 = mybir.dt.bfloat16
AF = mybir.ActivationFunctionType

NCORES = 8
B = 128     # batch rows per core
H = 512
I = 12
K4 = 4
T = 2048
XT_CHUNK = 64


def _build(T=T, xt_chunk=XT_CHUNK):
    nc = bacc.Bacc(None, target_bir_lowering=False, debug=False)

    xt_d = nc.dram_tensor("xt", [96, T, B], BF16, kind="ExternalInput")
    wmov_d = nc.dram_tensor("wmov", [128, K4, 1538], BF16, kind="ExternalInput")
    wih_d = nc.dram_tensor("wih", [128, 512], BF16, kind="ExternalInput")
    logits_d = nc.dram_tensor("logits", [B, T * 2], F32, kind="ExternalOutput")
    hout_d = nc.dram_tensor("hout", [B, H], F32, kind="ExternalOutput")

    assert T % xt_chunk == 0
    n_chunks = T // xt_chunk

    with tile.TileContext(nc) as tc:
        with ExitStack() as ctx:
            const = ctx.enter_context(tc.tile_pool(name="const", bufs=1))
            state = ctx.enter_context(tc.tile_pool(name="state", bufs=2))
            tmp = ctx.enter_context(tc.tile_pool(name="tmp", bufs=2))
            xtp = ctx.enter_context(tc.tile_pool(name="xtp", bufs=2))
            psum = ctx.enter_context(tc.tile_pool(name="psum", bufs=1, space="PSUM"))
            psum2 = ctx.enter_context(tc.tile_pool(name="psum2", bufs=2, space="PSUM"))

            w_sb = const.tile([128, K4, 1538], BF16, tag="w_sb")
            wih_sb = const.tile([128, 512], BF16, tag="wih_sb")
            ident = const.tile([128, 128], BF16, tag="ident")
            logit_sb = const.tile([B, T * 2], F32, tag="logit_sb")

            nc.sync.dma_start(out=w_sb[:], in_=wmov_d[:])
            nc.sync.dma_start(out=wih_sb[:], in_=wih_d[:])
            make_identity(nc, ident[:])

            h = state.tile([B, H], F32, tag="h")
            hT = state.tile([128, K4, 128], BF16, tag="hT")
            nc.vector.memset(h[:], 0.0)
            nc.vector.memset(hT[:], 0.0)

            p_lg = None
            for c_i in range(n_chunks):
                xt_sb = xtp.tile([96, xt_chunk * B], BF16, tag="xt")
                nc.sync.dma_start(
                    out=xt_sb[:],
                    in_=xt_d[:, c_i * xt_chunk : (c_i + 1) * xt_chunk, :],
                )
                for s in range(xt_chunk):
                    t = c_i * xt_chunk + s

                    p_r = psum2.tile([B, 512], F32, tag="p_r")
                    p_z = psum2.tile([B, 512], F32, tag="p_z")
                    p_n = psum.tile([B, 512], F32, tag="p_n")
                    p_in = psum.tile([B, 512], F32, tag="p_in")

                    for m, pg in enumerate((p_r, p_z, p_n)):
                        for k in range(K4):
                            nc.tensor.matmul(
                                pg[:],
                                hT[:, k, :],
                                w_sb[:, k, 512 * m : 512 * (m + 1)],
                                start=(k == 0),
                                stop=(k == 3 and m == 2),
                            )
                        if m < 2:
                            nc.tensor.matmul(
                                pg[:],
                                xt_sb[32 * m : 32 * m + I, s * B : (s + 1) * B],
                                wih_sb[32 * m : 32 * m + I, :],
                                start=False,
                                stop=True,
                            )
                    nc.tensor.matmul(
                        p_in[:],
                        xt_sb[64 : 64 + I, s * B : (s + 1) * B],
                        wih_sb[64 : 64 + I, :],
                        start=True,
                        stop=True,
                    )

                    # head for previous step's h (one step delayed)
                    if t > 0:
                        if (t - 1) % 256 == 0:
                            p_lg = psum.tile([B, 512], F32, tag="p_lg")
                        o = ((t - 1) % 256) * 2
                        for k in range(K4):
                            nc.tensor.matmul(
                                p_lg[:, o : o + 2],
                                hT[:, k, :],
                                w_sb[:, k, 1536:1538],
                                start=(k == 0),
                                stop=(k == 3),
                            )
                        if (t - 1) % 256 == 255:
                            base = (t - 1) - 255
                            nc.scalar.copy(
                                logit_sb[:, 2 * base : 2 * (base + 256)], p_lg[:]
                            )

                    r = tmp.tile([B, 512], F32, tag="r")
                    z1 = tmp.tile([B, 512], F32, tag="z1")
                    d = tmp.tile([B, 512], F32, tag="d")
                    n = tmp.tile([B, 512], F32, tag="n")
                    e = tmp.tile([B, 512], F32, tag="e")
                    f = tmp.tile([B, 512], F32, tag="f")
                    hb = tmp.tile([B, H], BF16, tag="hb")

                    nc.scalar.activation(r[:], p_r[:], AF.Sigmoid)
                    nc.scalar.activation(z1[:], p_z[:], AF.Sigmoid, scale=-1.0)
                    nc.vector.tensor_mul(d[:], r[:], p_n[:])
                    nc.vector.tensor_add(d[:], d[:], p_in[:])
                    nc.scalar.activation(n[:], d[:], AF.Tanh)

                    h_new = state.tile([B, H], F32, tag="h")
                    nc.vector.tensor_sub(e[:], n[:], h[:])
                    nc.vector.tensor_mul(f[:], z1[:], e[:])
                    nc.vector.tensor_add(h_new[:], h[:], f[:])
                    h = h_new

                    nc.vector.tensor_copy(hb[:], h[:])
                    hT_new = state.tile([128, K4, 128], BF16, tag="hT")
                    p_ht = psum.tile([128, K4 * 128], BF16, tag="p_ht")
                    for k in range(K4):
                        nc.tensor.transpose(
                            p_ht[:, 128 * k : 128 * (k + 1)],
                            hb[:, 128 * k : 128 * (k + 1)],
                            ident[:],
                        )
                    nc.vector.tensor_copy(hT_new[:], p_ht[:])
                    hT = hT_new

            # final head (logit for step T-1 uses final h)
            o = ((T - 1) % 256) * 2
            if (T - 1) % 256 == 0:
                p_lg = psum.tile([B, 512], F32, tag="p_lg")
            for k in range(K4):
                nc.tensor.matmul(
                    p_lg[:, o : o + 2],
                    hT[:, k, :],
                    w_sb[:, k, 1536:1538],
                    start=(k == 0),
                    stop=(k == 3),
                )
            base = (T - 1) - (T - 1) % 256
            nc.scalar.copy(
                logit_sb[:, 2 * base : 2 * T], p_lg[:, : ((T - 1) % 256 + 1) * 2]
            )

            nc.sync.dma_start(out=logits_d[:], in_=logit_sb[:])
            nc.sync.dma_start(out=hout_d[:], in_=h[:])

    nc.compile()
    return nc


def _prep_weights(w_ih, w_hh, w_out):
    wt = np.concatenate([w_hh, w_out], axis=0).T  # [512, 1538]
    wmov = np.ascontiguousarray(wt.reshape(K4, 128, 1538)).astype(ml_dtypes.bfloat16)
    wmov = np.ascontiguousarray(wmov.transpose(1, 0, 2))  # [128, k, 1538]
    wih = np.zeros((128, 512), dtype=ml_dtypes.bfloat16)
    wihT = w_ih.T  # [12, 1536]
    for m in range(3):
        wih[32 * m : 32 * m + I] = wihT[:, 512 * m : 512 * (m + 1)]
    return wmov, wih


def _prep_xt(history_c):
    """history_c [128, T, 12] fp32 -> xt [96, T, 128] bf16 (3 replicas at
    partition offsets 0/32/64 for PE row-group packing of the gi matmuls)."""
    Tc = history_c.shape[1]
    xt = np.zeros((96, Tc, B), dtype=ml_dtypes.bfloat16)
    xT = np.ascontiguousarray(history_c.transpose(2, 1, 0))
    for m in range(3):
        xt[32 * m : 32 * m + I] = xT
    return xt


_NC_CACHE = {}


def kernel(history, w_ih, w_hh, w_out, b_out):
    history = np.asarray(history, dtype=np.float32)
    w_ih = np.asarray(w_ih, dtype=np.float32)
    w_hh = np.asarray(w_hh, dtype=np.float32)
    w_out = np.asarray(w_out, dtype=np.float32)
    b_out = np.asarray(b_out, dtype=np.float32)

    Bf, Tf, If = history.shape
    assert (Bf, Tf, If) == (NCORES * B, T, I), (Bf, Tf, If)

    if "nc" not in _NC_CACHE:
        _NC_CACHE["nc"] = _build()
    nc = _NC_CACHE["nc"]

    wmov, wih = _prep_weights(w_ih, w_hh, w_out)
    in_maps = []
    for c in range(NCORES):
        in_maps.append({
            "xt": _prep_xt(history[c * B : (c + 1) * B]),
            "wmov": wmov,
            "wih": wih,
        })

    res = run_bass_kernel_spmd(nc, in_maps, core_ids=list(range(NCORES)))

    logits = np.concatenate(
        [res.results[c]["logits"].reshape(B, T, 2) for c in range(NCORES)], axis=0
    )
    h_final = np.concatenate(
        [res.results[c]["hout"] for c in range(NCORES)], axis=0
    )
    logits = logits + b_out[None, None, :]
    return logits.astype(np.float32), h_final.astype(np.float32)
